# revision 8
# baseline (speedup 1.0000x reference)
"""Trainium2 Bass kernel for ErnieImageAttention (non-causal MHA with per-head
RMSNorm on q/k + rotary embedding), tensor-parallel over heads on 8 NeuronCores.

Sharding: 16 heads / 8 cores = 2 heads per core. Each core computes its heads'
q/k/v projections, attention, and a partial output projection (row-parallel
Wo); the host sums the 8 partials and adds the bias.

Per-core dataflow (S=4096, D=2048, Hd=128, 2 local heads):
  phase 1 (per 128-row s-tile):
    q/k/v = hiddenT-chunk matmuls (bf16, N=256 = both heads) accumulated in
    PSUM; RMSNorm stats via Square-with-accum on ACT; RoPE via host-precomputed
    coefficient tables (g gains and the 1/sqrt(Hd) logit scale folded in; both
    r_q and r_k applied here); PE-transpose q/k 128x128 bf16 tiles into [d, s]
    layout; v kept [s, d] bf16.
  phase 2 (per 512-col q-block, per head, k in groups of 4 tiles):
    scoresT[k,q] = kT.T @ qT (bf16) into a 4-bank-wide PSUM region
    one wide Exp per group on ACT (amortizes the ~390ns ACTIVATE overhead)
    denominator: wide bf16 DVE adds, folded 2048->512, ones-matmul partition
    reduce, reciprocal_approx_fast, rank-1 ones matmul broadcast
    attn_T[d,q] = sum_k V[k,d]^T expT[k,q] accumulated in PSUM (bf16)
    normalize: attn_T * bcast(1/denom)
  phase 3 (inline per q-block): fin[s, :2048] = sum_h attnT_h.T @ WoT_h (f32r)

Softmax is max-subtraction-free: logits are ~N(0,1) by construction
(RMSNorm'd q/k, 1/sqrt(Hd) folded into q's rope tables).
"""

import numpy as np
import ml_dtypes

import concourse.bass as bass
import concourse.tile as tile
from concourse import bacc, mybir
from concourse import bass_utils
from concourse.masks import make_identity

F32 = mybir.dt.float32
F32R = mybir.dt.float32r
BF16 = mybir.dt.bfloat16
AX = mybir.AxisListType
AF = mybir.ActivationFunctionType

S = 4096
D = 2048
HD = 128
HEADS = 16
NCORES = 8
HLOC = HEADS // NCORES  # 2 heads per core
DLOC = HLOC * HD  # 256 local head dims
CH = D // 128  # 16 contraction chunks for projections
EPS = 1e-5
SCL = 1.0 / np.sqrt(HD)

QCOLS = 512  # q columns per attention block
KG = 4  # k tiles per wide-exp group


def build(nc, tc, io, s_len):
    st_n = s_len // 128  # s tiles
    qb_n = s_len // QCOLS  # q blocks
    qb_st = QCOLS // 128  # s tiles per q block
    kt_n = st_n  # k tiles
    kg_n = kt_n // KG  # k groups

    ht, wq, wk, wv, wo, cgq, sgq, cgk, sgk, out = (
        io["ht"], io["wq"], io["wk"], io["wv"], io["wo"],
        io["cgq"], io["sgq"], io["cgk"], io["sgk"], io["out"],
    )

    import contextlib

    with contextlib.ExitStack() as ctx:
        ctx.enter_context(nc.allow_low_precision(
            reason="bf16/f32r compute; values are O(1) and the rel-err "
                   "budget is 2e-2"))
        consts = ctx.enter_context(tc.tile_pool(name="consts", bufs=1))
        persist = ctx.enter_context(tc.tile_pool(name="persist", bufs=1))
        ht_pool = ctx.enter_context(tc.tile_pool(name="ht", bufs=2))
        cs_pool = ctx.enter_context(tc.tile_pool(name="cs", bufs=2))
        work = ctx.enter_context(tc.tile_pool(name="work", bufs=2))
        et_pool = ctx.enter_context(tc.tile_pool(name="et", bufs=3))
        at_pool = ctx.enter_context(tc.tile_pool(name="at", bufs=4))
        araw_pool = ctx.enter_context(tc.tile_pool(name="araw", bufs=2))
        acc_pool = ctx.enter_context(tc.tile_pool(name="acc", bufs=2))
        rc_pool = ctx.enter_context(tc.tile_pool(name="rc", bufs=2))
        fin_pool = ctx.enter_context(tc.tile_pool(name="fin", bufs=4))

        # constants
        ident = consts.tile([128, 128], BF16)
        make_identity(nc, ident[:])
        ones_f32 = consts.tile([128, 1], F32)
        nc.vector.memset(ones_f32[:], 1.0)
        ones_col = consts.tile([128, 1], F32R)
        nc.vector.tensor_copy(ones_col[:], ones_f32[:])
        ones_row_f32 = consts.tile([1, 128], F32)
        nc.vector.memset(ones_row_f32[:], 1.0)
        ones_row = consts.tile([1, 128], F32R)
        nc.vector.tensor_copy(ones_row[:], ones_row_f32[:])
        eps_t = consts.tile([128, 1], F32)
        nc.vector.memset(eps_t[:], EPS)

        wq_sb = consts.tile([128, CH, DLOC], BF16)
        nc.sync.dma_start(out=wq_sb[:], in_=wq)
        wk_sb = consts.tile([128, CH, DLOC], BF16)
        nc.sync.dma_start(out=wk_sb[:], in_=wk)
        wv_sb = consts.tile([128, CH, DLOC], BF16)
        nc.sync.dma_start(out=wv_sb[:], in_=wv)
        wo_sb = consts.tile([128, HLOC, D], F32R)
        nc.sync.dma_start(out=wo_sb[:], in_=wo)

        # persistent per-head transposed q/k and v
        qT_sb = persist.tile([128, HLOC, st_n, 128], BF16)
        kT_sb = persist.tile([128, HLOC, st_n, 128], BF16)
        v_sb = persist.tile([128, st_n, DLOC], BF16)

        # ---------------- phase 1: projections + norm + rope + transpose ----
        with tc.tile_pool(name="ps1", bufs=2, space="PSUM") as ps1:
            for st in range(st_n):
                ss = slice(st * 128, (st + 1) * 128)
                ht_t = ht_pool.tile([128, CH, 128], BF16, tag="ht")
                nc.sync.dma_start(out=ht_t[:], in_=ht[st])
                cgq_t = cs_pool.tile([128, DLOC], F32, tag="cgq")
                nc.sync.dma_start(out=cgq_t[:], in_=cgq[ss, :])
                sgq_t = cs_pool.tile([128, DLOC], F32, tag="sgq")
                nc.sync.dma_start(out=sgq_t[:], in_=sgq[ss, :])
                cgk_t = cs_pool.tile([128, DLOC], F32, tag="cgk")
                nc.sync.dma_start(out=cgk_t[:], in_=cgk[ss, :])
                sgk_t = cs_pool.tile([128, DLOC], F32, tag="sgk")
                nc.sync.dma_start(out=sgk_t[:], in_=sgk[ss, :])

                pq = ps1.tile([128, DLOC], F32, tag="pq")
                pk = ps1.tile([128, DLOC], F32, tag="pk")
                pv = ps1.tile([128, DLOC], F32, tag="pv")
                for c in range(CH):
                    lhs = ht_t[:, c, :]
                    nc.tensor.matmul(pq[:], lhs, wq_sb[:, c, :],
                                     start=(c == 0), stop=(c == CH - 1))
                    nc.tensor.matmul(pk[:], lhs, wk_sb[:, c, :],
                                     start=(c == 0), stop=(c == CH - 1))
                    nc.tensor.matmul(pv[:], lhs, wv_sb[:, c, :],
                                     start=(c == 0), stop=(c == CH - 1))

                # v: PSUM -> SBUF bf16
                nc.scalar.copy(v_sb[:, st, :], pv[:])

                # rms stats: Square with free-dim accumulation -> sum(q^2)
                varq = work.tile([128, HLOC], F32, tag="varq")
                vark = work.tile([128, HLOC], F32, tag="vark")
                sqd = work.tile([128, HD], F32, tag="sqd")  # dump
                for h in range(HLOC):
                    hs = slice(h * HD, (h + 1) * HD)
                    nc.scalar.activation(sqd[:], pq[:, hs], AF.Square,
                                         accum_out=varq[:, h:h + 1])
                    nc.scalar.activation(sqd[:], pk[:, hs], AF.Square,
                                         accum_out=vark[:, h:h + 1])
                sigq = work.tile([128, HLOC], F32, tag="sigq")
                nc.scalar.activation(sigq[:], varq[:], AF.Sqrt,
                                     bias=eps_t[:], scale=1.0 / HD)
                rq = work.tile([128, HLOC], F32, tag="rq")
                nc.vector.reciprocal_approx_fast(rq[:], sigq[:])
                sigk = work.tile([128, HLOC], F32, tag="sigk")
                nc.scalar.activation(sigk[:], vark[:], AF.Sqrt,
                                     bias=eps_t[:], scale=1.0 / HD)
                rk = work.tile([128, HLOC], F32, tag="rk")
                nc.vector.reciprocal_approx_fast(rk[:], sigk[:])

                # rope: out = (r*x) . CG + shift64(r*x) . SG   (per tensor)
                for name, psrc, r, cg, sg, dstT in (
                    ("q", pq, rq, cgq_t, sgq_t, qT_sb),
                    ("k", pk, rk, cgk_t, sgk_t, kT_sb),
                ):
                    xs = work.tile([128, DLOC], F32, tag=f"xs{name}")
                    for h in range(HLOC):
                        hs = slice(h * HD, (h + 1) * HD)
                        nc.vector.tensor_scalar_mul(xs[:, hs], psrc[:, hs],
                                                    r[:, h:h + 1])
                    m1 = work.tile([128, DLOC], F32, tag=f"m1{name}")
                    nc.vector.tensor_mul(m1[:], xs[:], cg[:])
                    m2 = work.tile([128, DLOC], F32, tag=f"m2{name}")
                    x4 = xs[:].rearrange("p (h t u) -> p h t u", h=HLOC, t=2)
                    m4 = m2[:].rearrange("p (h t u) -> p h t u", h=HLOC, t=2)
                    g4 = sg[:].rearrange("p (h t u) -> p h t u", h=HLOC, t=2)
                    nc.vector.tensor_mul(m4[:, :, 0, :], x4[:, :, 1, :],
                                         g4[:, :, 0, :])
                    nc.vector.tensor_mul(m4[:, :, 1, :], x4[:, :, 0, :],
                                         g4[:, :, 1, :])
                    xa = work.tile([128, DLOC], BF16, tag=f"xa{name}")
                    nc.vector.tensor_add(xa[:], m1[:], m2[:])
                    for h in range(HLOC):
                        hs = slice(h * HD, (h + 1) * HD)
                        ptp = ps1.tile([128, 128], BF16, tag="ptp")
                        nc.tensor.transpose(ptp[:], xa[:, hs], ident[:])
                        nc.scalar.copy(dstT[:, h, st, :], ptp[:])

        # ---------------- phase 2+3: attention + output projection ----------
        with (
            tc.tile_pool(name="psA", bufs=1, space="PSUM") as psA,
            tc.tile_pool(name="psB", bufs=2, space="PSUM") as psB,
        ):
            for qb in range(qb_n):
                ats = []
                for h in range(HLOC):
                    acc = acc_pool.tile([128, KG, QCOLS], BF16, tag="acc")
                    po = psB.tile([128, QCOLS], F32, tag="po")
                    q_rhs = qT_sb[:, h, qb * qb_st:(qb + 1) * qb_st, :]
                    for g in range(kg_n):
                        sc = psA.tile([128, KG, QCOLS], F32, tag="sc")
                        for j in range(KG):
                            nc.tensor.matmul(sc[:, j, :],
                                             kT_sb[:, h, g * KG + j, :],
                                             q_rhs, start=True, stop=True)
                        et = et_pool.tile([128, KG, QCOLS], BF16, tag="et")
                        nc.scalar.activation(et[:], sc[:], AF.Exp)
                        if g == 0:
                            nc.vector.tensor_copy(acc[:], et[:])
                        else:
                            nc.vector.tensor_add(acc[:], acc[:], et[:])
                        for j in range(KG):
                            kt = g * KG + j
                            nc.tensor.matmul(po[:],
                                             v_sb[:, kt, h * HD:(h + 1) * HD],
                                             et[:, j, :], start=(kt == 0),
                                             stop=(kt == kt_n - 1))
                    # fold 4 accumulator lanes -> 1, f32r
                    nc.vector.tensor_add(acc[:, 0, :], acc[:, 0, :],
                                         acc[:, 1, :])
                    nc.vector.tensor_add(acc[:, 2, :], acc[:, 2, :],
                                         acc[:, 3, :])
                    accf = acc_pool.tile([128, QCOLS], F32R, tag="accf")
                    nc.vector.tensor_add(accf[:], acc[:, 0, :], acc[:, 2, :])
                    # denominator: partition-reduce, 1/x, broadcast
                    pd = psA.tile([1, QCOLS], F32, tag="pd")
                    nc.tensor.matmul(pd[:], ones_col[:], accf[:],
                                     start=True, stop=True)
                    rsb = rc_pool.tile([1, QCOLS], F32, tag="rsb")
                    nc.vector.reciprocal_approx_fast(rsb[:], pd[:])
                    rsr = rc_pool.tile([1, QCOLS], F32R, tag="rsr")
                    nc.vector.tensor_copy(rsr[:], rsb[:])
                    pb = psA.tile([128, QCOLS], F32, tag="pb")
                    nc.tensor.matmul(pb[:], ones_row[:], rsr[:],
                                     start=True, stop=True)
                    araw = araw_pool.tile([128, QCOLS], F32R, tag="araw")
                    nc.scalar.copy(araw[:], po[:])
                    at = at_pool.tile([128, QCOLS], F32R, tag="at")
                    nc.vector.tensor_mul(at[:], araw[:], pb[:])
                    ats.append(at)

                # output projection for this q block
                for sti in range(qb_st):
                    st = qb * qb_st + sti
                    sl = slice(sti * 128, (sti + 1) * 128)
                    for nchunk in range(D // 512):
                        ns = slice(nchunk * 512, (nchunk + 1) * 512)
                        pf = psB.tile([128, 512], F32, tag="po")
                        for h in range(HLOC):
                            nc.tensor.matmul(pf[:], ats[h][:, sl],
                                             wo_sb[:, h, ns],
                                             start=(h == 0),
                                             stop=(h == HLOC - 1))
                        fin = fin_pool.tile([128, 512], F32R, tag="fin")
                        nc.vector.tensor_copy(fin[:], pf[:])
                        nc.sync.dma_start(
                            out=out[st * 128:(st + 1) * 128, ns], in_=fin[:])


def build_program(s_len=S):
    nc = bacc.Bacc("TRN2", target_bir_lowering=False, debug=False,
                   enable_asserts=False)
    st_n = s_len // 128
    io = {
        "ht": nc.dram_tensor("ht", [st_n, 128, CH, 128], BF16,
                             kind="ExternalInput").ap(),
        "wq": nc.dram_tensor("wq", [128, CH, DLOC], BF16,
                             kind="ExternalInput").ap(),
        "wk": nc.dram_tensor("wk", [128, CH, DLOC], BF16,
                             kind="ExternalInput").ap(),
        "wv": nc.dram_tensor("wv", [128, CH, DLOC], BF16,
                             kind="ExternalInput").ap(),
        "wo": nc.dram_tensor("wo", [128, HLOC, D], F32R,
                             kind="ExternalInput").ap(),
        "cgq": nc.dram_tensor("cgq", [s_len, DLOC], F32,
                              kind="ExternalInput").ap(),
        "sgq": nc.dram_tensor("sgq", [s_len, DLOC], F32,
                              kind="ExternalInput").ap(),
        "cgk": nc.dram_tensor("cgk", [s_len, DLOC], F32,
                              kind="ExternalInput").ap(),
        "sgk": nc.dram_tensor("sgk", [s_len, DLOC], F32,
                              kind="ExternalInput").ap(),
        "out": nc.dram_tensor("out", [s_len, D], F32R,
                              kind="ExternalOutput").ap(),
    }
    with tile.TileContext(nc) as tc:
        build(nc, tc, io, s_len)
    nc.compile()
    return nc


def prep_inputs(inputs, s_len=S):
    """Host-side preprocessing: transposed/tiled bf16 layouts + rope
    coefficient tables (g gains and the 1/sqrt(Hd) scale folded in,
    duplicated per local head for full-width elementwise ops)."""
    bf16 = ml_dtypes.bfloat16
    hs = np.asarray(inputs["hidden_states"], np.float32).reshape(s_len, D)
    st_n = s_len // 128
    ht = np.ascontiguousarray(
        hs.reshape(st_n, 128, CH, 128).transpose(0, 3, 2, 1)).astype(bf16)

    fc = np.asarray(inputs["freqs_cis"], np.float32).reshape(s_len, HD)
    cos = np.cos(fc)
    sin = np.sin(fc)
    gq = np.asarray(inputs["gq"], np.float32)
    gk = np.asarray(inputs["gk"], np.float32)

    def coef(g, scale):
        cg = cos * g[None, :] * scale
        sg = np.empty_like(sin)
        sg[:, :64] = -sin[:, :64] * g[None, 64:] * scale
        sg[:, 64:] = sin[:, 64:] * g[None, :64] * scale
        cg2 = np.ascontiguousarray(np.tile(cg, (1, HLOC)))
        sg2 = np.ascontiguousarray(np.tile(sg, (1, HLOC)))
        return cg2, sg2

    cgq, sgq = coef(gq, SCL)
    cgk, sgk = coef(gk, 1.0)

    Wq = np.asarray(inputs["Wq"], np.float32)
    Wk = np.asarray(inputs["Wk"], np.float32)
    Wv = np.asarray(inputs["Wv"], np.float32)
    Wo = np.asarray(inputs["Wo"], np.float32)

    in_maps = []
    for c in range(NCORES):
        cols = slice(DLOC * c, DLOC * (c + 1))
        wq_c = np.ascontiguousarray(
            Wq[cols, :].T.reshape(CH, 128, DLOC).transpose(1, 0, 2)).astype(bf16)
        wk_c = np.ascontiguousarray(
            Wk[cols, :].T.reshape(CH, 128, DLOC).transpose(1, 0, 2)).astype(bf16)
        wv_c = np.ascontiguousarray(
            Wv[cols, :].T.reshape(CH, 128, DLOC).transpose(1, 0, 2)).astype(bf16)
        wo_c = np.ascontiguousarray(
            Wo[:, cols].T.reshape(HLOC, 128, D).transpose(1, 0, 2))
        in_maps.append({
            "ht": ht, "wq": wq_c, "wk": wk_c, "wv": wv_c, "wo": wo_c,
            "cgq": cgq, "sgq": sgq, "cgk": cgk, "sgk": sgk,
        })
    return in_maps


_CACHE = {}


def run_full(inputs, trace=False, **kw):
    if "nc" not in _CACHE:
        _CACHE["nc"] = build_program(S)
    nc = _CACHE["nc"]
    in_maps = prep_inputs(inputs, S)
    res = bass_utils.run_bass_kernel_spmd(
        nc, in_maps, core_ids=list(range(NCORES)), trace=trace, **kw)
    total = res.results[0]["out"].astype(np.float64)
    for c in range(1, NCORES):
        total += res.results[c]["out"]
    total += np.asarray(inputs["bo"], np.float64)[None, :]
    out = total.astype(np.float32).reshape(1, S, D)
    return out, res


def kernel(**inputs):
    out, _ = run_full(inputs, trace=False)
    return out


# revision 12
# speedup vs baseline: 1.2783x; 1.2783x over previous
"""Trainium2 Bass kernel for ErnieImageAttention (non-causal MHA with per-head
RMSNorm on q/k + rotary embedding), tensor-parallel over heads on 8 NeuronCores.

Sharding: 16 heads / 8 cores = 2 heads per core. Each core computes its heads'
q/k/v projections, attention, and a partial output projection (row-parallel
Wo); the host sums the 8 partials and adds the bias.

Per-core dataflow (S=4096, D=2048, Hd=128, 2 local heads):
  phase 1 (per 128-row s-tile):
    q/k/v = hiddenT-chunk matmuls (bf16, N=256 = both heads) accumulated in
    PSUM; RMSNorm stats via Square-with-accum on ACT; RoPE via host-precomputed
    coefficient tables (g gains and the 1/sqrt(Hd) logit scale folded in; both
    r_q and r_k applied here); PE-transpose q/k 128x128 bf16 tiles into [d, s]
    layout; v kept [s, d] bf16.
  phase 2 (per 512-col q-block, per head, k in groups of 4 tiles):
    scoresT[k,q] = kT.T @ qT (bf16) into a 4-bank-wide PSUM region
    one wide Exp per group on ACT (amortizes the ~390ns ACTIVATE overhead)
    denominator: wide bf16 DVE adds, folded 2048->512, ones-matmul partition
    reduce, reciprocal_approx_fast, rank-1 ones matmul broadcast
    attn_T[d,q] = sum_k V[k,d]^T expT[k,q] accumulated in PSUM (bf16)
    normalize: attn_T * bcast(1/denom)
  phase 3 (inline per q-block): fin[s, :2048] = sum_h attnT_h.T @ WoT_h (f32r)

Softmax is max-subtraction-free: logits are ~N(0,1) by construction
(RMSNorm'd q/k, 1/sqrt(Hd) folded into q's rope tables).
"""

import numpy as np
import ml_dtypes

import concourse.bass as bass
import concourse.tile as tile
from concourse import bacc, mybir
from concourse import bass_utils
from concourse.masks import make_identity

F32 = mybir.dt.float32
F32R = mybir.dt.float32r
BF16 = mybir.dt.bfloat16
AX = mybir.AxisListType
AF = mybir.ActivationFunctionType

S = 4096
D = 2048
HD = 128
HEADS = 16
NCORES = 8
HLOC = HEADS // NCORES  # 2 heads per core
DLOC = HLOC * HD  # 256 local head dims
CH = D // 128  # 16 contraction chunks for projections
EPS = 1e-5
SCL = 1.0 / np.sqrt(HD)

QCOLS = 512  # q columns per attention block
KG = 4  # k tiles per wide-exp group


def build(nc, tc, io, s_len):
    st_n = s_len // 128  # s tiles
    qb_n = s_len // QCOLS  # q blocks
    qb_st = QCOLS // 128  # s tiles per q block
    kt_n = st_n  # k tiles
    kg_n = kt_n // KG  # k groups

    ht, wq, wk, wv, wo, cgq, sgq, cgk, sgk, out = (
        io["ht"], io["wq"], io["wk"], io["wv"], io["wo"],
        io["cgq"], io["sgq"], io["cgk"], io["sgk"], io["out"],
    )

    import contextlib

    with contextlib.ExitStack() as ctx:
        ctx.enter_context(nc.allow_low_precision(
            reason="bf16/f32r compute; values are O(1) and the rel-err "
                   "budget is 2e-2"))
        consts = ctx.enter_context(tc.tile_pool(name="consts", bufs=1))
        persist = ctx.enter_context(tc.tile_pool(name="persist", bufs=1))
        ht_pool = ctx.enter_context(tc.tile_pool(name="ht", bufs=2))
        cs_pool = ctx.enter_context(tc.tile_pool(name="cs", bufs=2))
        work = ctx.enter_context(tc.tile_pool(name="work", bufs=2))
        et_pool = ctx.enter_context(tc.tile_pool(name="et", bufs=3))
        at_pool = ctx.enter_context(tc.tile_pool(name="at", bufs=4))
        araw_pool = ctx.enter_context(tc.tile_pool(name="araw", bufs=2))
        acc_pool = ctx.enter_context(tc.tile_pool(name="acc", bufs=2))
        rc_pool = ctx.enter_context(tc.tile_pool(name="rc", bufs=1))
        fin_pool = ctx.enter_context(tc.tile_pool(name="fin", bufs=4))

        # constants
        ident = consts.tile([128, 128], BF16)
        make_identity(nc, ident[:])
        ones_f32 = consts.tile([128, 1], F32)
        nc.vector.memset(ones_f32[:], 1.0)
        ones_col = consts.tile([128, 1], F32R)
        nc.vector.tensor_copy(ones_col[:], ones_f32[:])
        ones_col_bf = consts.tile([128, 1], BF16)
        nc.vector.tensor_copy(ones_col_bf[:], ones_f32[:])
        ones_row_f32 = consts.tile([1, 128], F32)
        nc.vector.memset(ones_row_f32[:], 1.0)
        ones_row = consts.tile([1, 128], F32R)
        nc.vector.tensor_copy(ones_row[:], ones_row_f32[:])
        eps_t = consts.tile([128, 1], F32)
        nc.vector.memset(eps_t[:], EPS)

        wq_sb = consts.tile([128, CH, DLOC], BF16)
        nc.sync.dma_start(out=wq_sb[:], in_=wq)
        wk_sb = consts.tile([128, CH, DLOC], BF16)
        nc.sync.dma_start(out=wk_sb[:], in_=wk)
        wv_sb = consts.tile([128, CH, DLOC], BF16)
        nc.sync.dma_start(out=wv_sb[:], in_=wv)
        wo_sb = consts.tile([128, HLOC, D], F32R)
        nc.sync.dma_start(out=wo_sb[:], in_=wo)

        # persistent per-head transposed q/k and v
        qT_sb = persist.tile([128, HLOC, st_n, 128], BF16)
        kT_sb = persist.tile([128, HLOC, st_n, 128], BF16)
        v_sb = persist.tile([128, st_n, DLOC], BF16)

        # ---------------- phase 1: projections + norm + rope + transpose ----
        with tc.tile_pool(name="ps1", bufs=2, space="PSUM") as ps1:
            for st in range(st_n):
                ss = slice(st * 128, (st + 1) * 128)
                ht_t = ht_pool.tile([128, CH, 128], BF16, tag="ht")
                nc.sync.dma_start(out=ht_t[:], in_=ht[st])
                cgq_t = cs_pool.tile([128, DLOC], F32, tag="cgq")
                nc.sync.dma_start(out=cgq_t[:], in_=cgq[ss, :])
                sgq_t = cs_pool.tile([128, DLOC], F32, tag="sgq")
                nc.sync.dma_start(out=sgq_t[:], in_=sgq[ss, :])
                cgk_t = cs_pool.tile([128, DLOC], F32, tag="cgk")
                nc.sync.dma_start(out=cgk_t[:], in_=cgk[ss, :])
                sgk_t = cs_pool.tile([128, DLOC], F32, tag="sgk")
                nc.sync.dma_start(out=sgk_t[:], in_=sgk[ss, :])

                pq = ps1.tile([128, DLOC], F32, tag="pq")
                pk = ps1.tile([128, DLOC], F32, tag="pk")
                pv = ps1.tile([128, DLOC], F32, tag="pv")
                for c in range(CH):
                    lhs = ht_t[:, c, :]
                    nc.tensor.matmul(pq[:], lhs, wq_sb[:, c, :],
                                     start=(c == 0), stop=(c == CH - 1))
                    nc.tensor.matmul(pk[:], lhs, wk_sb[:, c, :],
                                     start=(c == 0), stop=(c == CH - 1))
                    nc.tensor.matmul(pv[:], lhs, wv_sb[:, c, :],
                                     start=(c == 0), stop=(c == CH - 1))

                # v: PSUM -> SBUF bf16
                nc.scalar.copy(v_sb[:, st, :], pv[:])

                # rms stats: Square with free-dim accumulation -> sum(q^2)
                varq = work.tile([128, HLOC], F32, tag="varq")
                vark = work.tile([128, HLOC], F32, tag="vark")
                sqd = work.tile([128, HD], F32, tag="sqd")  # dump
                for h in range(HLOC):
                    hs = slice(h * HD, (h + 1) * HD)
                    nc.scalar.activation(sqd[:], pq[:, hs], AF.Square,
                                         accum_out=varq[:, h:h + 1])
                    nc.scalar.activation(sqd[:], pk[:, hs], AF.Square,
                                         accum_out=vark[:, h:h + 1])
                sigq = work.tile([128, HLOC], F32, tag="sigq")
                nc.scalar.activation(sigq[:], varq[:], AF.Sqrt,
                                     bias=eps_t[:], scale=1.0 / HD)
                rq = work.tile([128, HLOC], F32, tag="rq")
                nc.vector.reciprocal_approx_fast(rq[:], sigq[:])
                sigk = work.tile([128, HLOC], F32, tag="sigk")
                nc.scalar.activation(sigk[:], vark[:], AF.Sqrt,
                                     bias=eps_t[:], scale=1.0 / HD)
                rk = work.tile([128, HLOC], F32, tag="rk")
                nc.vector.reciprocal_approx_fast(rk[:], sigk[:])

                # rope: out = (r*x) . CG + shift64(r*x) . SG   (per tensor)
                for name, psrc, r, cg, sg, dstT in (
                    ("q", pq, rq, cgq_t, sgq_t, qT_sb),
                    ("k", pk, rk, cgk_t, sgk_t, kT_sb),
                ):
                    xs = work.tile([128, DLOC], F32, tag=f"xs{name}")
                    for h in range(HLOC):
                        hs = slice(h * HD, (h + 1) * HD)
                        nc.vector.tensor_scalar_mul(xs[:, hs], psrc[:, hs],
                                                    r[:, h:h + 1])
                    m1 = work.tile([128, DLOC], F32, tag=f"m1{name}")
                    nc.vector.tensor_mul(m1[:], xs[:], cg[:])
                    m2 = work.tile([128, DLOC], F32, tag=f"m2{name}")
                    x4 = xs[:].rearrange("p (h t u) -> p h t u", h=HLOC, t=2)
                    m4 = m2[:].rearrange("p (h t u) -> p h t u", h=HLOC, t=2)
                    g4 = sg[:].rearrange("p (h t u) -> p h t u", h=HLOC, t=2)
                    nc.vector.tensor_mul(m4[:, :, 0, :], x4[:, :, 1, :],
                                         g4[:, :, 0, :])
                    nc.vector.tensor_mul(m4[:, :, 1, :], x4[:, :, 0, :],
                                         g4[:, :, 1, :])
                    xa = work.tile([128, DLOC], BF16, tag=f"xa{name}")
                    nc.vector.tensor_add(xa[:], m1[:], m2[:])
                    for h in range(HLOC):
                        hs = slice(h * HD, (h + 1) * HD)
                        ptp = ps1.tile([128, 128], BF16, tag="ptp")
                        nc.tensor.transpose(ptp[:], xa[:, hs], ident[:])
                        nc.scalar.copy(dstT[:, h, st, :], ptp[:])

        # ---------------- phase 2+3: attention + output projection ----------
        # Ping-pong wide score buffers (4-bank A / 2-bank B) with software
        # pipelining: group g+1's score matmuls are emitted BEFORE group g's
        # PV matmuls so the in-order PE queue never stalls behind ACT's exp.
        with (
            tc.tile_pool(name="psA", bufs=1, space="PSUM") as psA,
            tc.tile_pool(name="psB", bufs=2, space="PSUM") as psB,
        ):
            groups = []  # (kt0, glen, tag)
            kt0 = 0
            gi = 0
            while kt0 < kt_n:
                tag = "scA" if gi % 2 == 0 else "scB"
                cap = 4 if tag == "scA" else 2
                glen = min(cap, kt_n - kt0)
                groups.append((kt0, glen, tag))
                kt0 += glen
                gi += 1

            for qb in range(qb_n):
                ats = []
                lanesA = max((g[1] for g in groups if g[2] == "scA"),
                             default=0)
                lanesB = max((g[1] for g in groups if g[2] == "scB"),
                             default=0)
                for h in range(HLOC):
                    accA = acc_pool.tile([128, 4, QCOLS], BF16, tag="accA")
                    accB = None
                    if lanesB:
                        accB = acc_pool.tile([128, 2, QCOLS], BF16,
                                             tag="accB", name="accB")
                    first = {"scA": True, "scB": True}
                    po = psB.tile([128, QCOLS], F32, tag="po")
                    q_rhs = qT_sb[:, h, qb * qb_st:(qb + 1) * qb_st, :]

                    def flush(et, kt0, glen, tag):
                        acc = accA if tag == "scA" else accB
                        if first[tag]:
                            nc.vector.tensor_copy(acc[:, 0:glen, :],
                                                  et[:, 0:glen, :])
                            first[tag] = False
                        else:
                            nc.vector.tensor_add(acc[:, 0:glen, :],
                                                 acc[:, 0:glen, :],
                                                 et[:, 0:glen, :])
                        for j in range(glen):
                            kt = kt0 + j
                            nc.tensor.matmul(po[:],
                                             v_sb[:, kt, h * HD:(h + 1) * HD],
                                             et[:, j, :], start=(kt == 0),
                                             stop=(kt == kt_n - 1))

                    prev = None
                    for (kt0, glen, tag) in groups:
                        width = 4 if tag == "scA" else 2
                        sc = psA.tile([128, width, QCOLS], F32, tag=tag)
                        for j in range(glen):
                            nc.tensor.matmul(sc[:, j, :],
                                             kT_sb[:, h, kt0 + j, :],
                                             q_rhs, start=True, stop=True)
                        et = et_pool.tile([128, 4, QCOLS], BF16, tag="et")
                        nc.scalar.activation(et[:, 0:glen, :],
                                             sc[:, 0:glen, :], AF.Exp)
                        if prev is not None:
                            flush(*prev)
                        prev = (et, kt0, glen, tag)
                    flush(*prev)

                    # denominator: lane ones-matmuls accumulate in PSUM,
                    # 1/x, rank-1 broadcast (pd/pb share one bank in scB slot)
                    pdb = psA.tile([128, 2, QCOLS], F32, tag="scB")
                    pd = pdb[0:1, 0, :]
                    lanes = [accA[:, j, :] for j in range(lanesA)]
                    if accB is not None:
                        lanes += [accB[:, j, :] for j in range(lanesB)]
                    for i, lane in enumerate(lanes):
                        nc.tensor.matmul(pd, ones_col_bf[:], lane,
                                         start=(i == 0),
                                         stop=(i == len(lanes) - 1))
                    rsb = rc_pool.tile([1, QCOLS], F32, tag="rsb")
                    nc.vector.reciprocal_approx_fast(rsb[:], pd)
                    rsr = rc_pool.tile([1, QCOLS], F32R, tag="rsr")
                    nc.vector.tensor_copy(rsr[:], rsb[:])
                    nc.tensor.matmul(pdb[:, 0, :], ones_row[:], rsr[:],
                                     start=True, stop=True)
                    araw = araw_pool.tile([128, QCOLS], F32R, tag="araw")
                    nc.scalar.copy(araw[:], po[:])
                    at = at_pool.tile([128, QCOLS], F32R, tag="at")
                    nc.vector.tensor_mul(at[:], araw[:], pdb[:, 0, :])
                    ats.append(at)

                # output projection for this q block
                pfi = 0
                for sti in range(qb_st):
                    st = qb * qb_st + sti
                    sl = slice(sti * 128, (sti + 1) * 128)
                    for nchunk in range(D // 512):
                        ns = slice(nchunk * 512, (nchunk + 1) * 512)
                        if pfi % 2 == 0:
                            pft = psA.tile([128, 2, QCOLS], F32, tag="scB",
                                           name=f"pfA{pfi}")
                            pf = pft[:, 0, :]
                        else:
                            pft = psB.tile([128, QCOLS], F32, tag="po",
                                           name=f"pfB{pfi}")
                            pf = pft[:]
                        pfi += 1
                        for h in range(HLOC):
                            nc.tensor.matmul(pf, ats[h][:, sl],
                                             wo_sb[:, h, ns],
                                             start=(h == 0),
                                             stop=(h == HLOC - 1))
                        fin = fin_pool.tile([128, 512], F32R, tag="fin")
                        nc.any.tensor_copy(fin[:], pf)
                        nc.sync.dma_start(
                            out=out[st * 128:(st + 1) * 128, ns], in_=fin[:])


def build_program(s_len=S):
    nc = bacc.Bacc("TRN2", target_bir_lowering=False, debug=False,
                   enable_asserts=False)
    st_n = s_len // 128
    io = {
        "ht": nc.dram_tensor("ht", [st_n, 128, CH, 128], BF16,
                             kind="ExternalInput").ap(),
        "wq": nc.dram_tensor("wq", [128, CH, DLOC], BF16,
                             kind="ExternalInput").ap(),
        "wk": nc.dram_tensor("wk", [128, CH, DLOC], BF16,
                             kind="ExternalInput").ap(),
        "wv": nc.dram_tensor("wv", [128, CH, DLOC], BF16,
                             kind="ExternalInput").ap(),
        "wo": nc.dram_tensor("wo", [128, HLOC, D], F32R,
                             kind="ExternalInput").ap(),
        "cgq": nc.dram_tensor("cgq", [s_len, DLOC], F32,
                              kind="ExternalInput").ap(),
        "sgq": nc.dram_tensor("sgq", [s_len, DLOC], F32,
                              kind="ExternalInput").ap(),
        "cgk": nc.dram_tensor("cgk", [s_len, DLOC], F32,
                              kind="ExternalInput").ap(),
        "sgk": nc.dram_tensor("sgk", [s_len, DLOC], F32,
                              kind="ExternalInput").ap(),
        "out": nc.dram_tensor("out", [s_len, D], F32R,
                              kind="ExternalOutput").ap(),
    }
    with tile.TileContext(nc) as tc:
        build(nc, tc, io, s_len)
    nc.compile()
    return nc


def prep_inputs(inputs, s_len=S):
    """Host-side preprocessing: transposed/tiled bf16 layouts + rope
    coefficient tables (g gains and the 1/sqrt(Hd) scale folded in,
    duplicated per local head for full-width elementwise ops)."""
    bf16 = ml_dtypes.bfloat16
    hs = np.asarray(inputs["hidden_states"], np.float32).reshape(s_len, D)
    st_n = s_len // 128
    ht = np.ascontiguousarray(
        hs.reshape(st_n, 128, CH, 128).transpose(0, 3, 2, 1)).astype(bf16)

    fc = np.asarray(inputs["freqs_cis"], np.float32).reshape(s_len, HD)
    cos = np.cos(fc)
    sin = np.sin(fc)
    gq = np.asarray(inputs["gq"], np.float32)
    gk = np.asarray(inputs["gk"], np.float32)

    def coef(g, scale):
        cg = cos * g[None, :] * scale
        sg = np.empty_like(sin)
        sg[:, :64] = -sin[:, :64] * g[None, 64:] * scale
        sg[:, 64:] = sin[:, 64:] * g[None, :64] * scale
        cg2 = np.ascontiguousarray(np.tile(cg, (1, HLOC)))
        sg2 = np.ascontiguousarray(np.tile(sg, (1, HLOC)))
        return cg2, sg2

    cgq, sgq = coef(gq, SCL)
    cgk, sgk = coef(gk, 1.0)

    Wq = np.asarray(inputs["Wq"], np.float32)
    Wk = np.asarray(inputs["Wk"], np.float32)
    Wv = np.asarray(inputs["Wv"], np.float32)
    Wo = np.asarray(inputs["Wo"], np.float32)

    in_maps = []
    for c in range(NCORES):
        cols = slice(DLOC * c, DLOC * (c + 1))
        wq_c = np.ascontiguousarray(
            Wq[cols, :].T.reshape(CH, 128, DLOC).transpose(1, 0, 2)).astype(bf16)
        wk_c = np.ascontiguousarray(
            Wk[cols, :].T.reshape(CH, 128, DLOC).transpose(1, 0, 2)).astype(bf16)
        wv_c = np.ascontiguousarray(
            Wv[cols, :].T.reshape(CH, 128, DLOC).transpose(1, 0, 2)).astype(bf16)
        wo_c = np.ascontiguousarray(
            Wo[:, cols].T.reshape(HLOC, 128, D).transpose(1, 0, 2))
        in_maps.append({
            "ht": ht, "wq": wq_c, "wk": wk_c, "wv": wv_c, "wo": wo_c,
            "cgq": cgq, "sgq": sgq, "cgk": cgk, "sgk": sgk,
        })
    return in_maps


_CACHE = {}


def run_full(inputs, trace=False, **kw):
    if "nc" not in _CACHE:
        _CACHE["nc"] = build_program(S)
    nc = _CACHE["nc"]
    in_maps = prep_inputs(inputs, S)
    res = bass_utils.run_bass_kernel_spmd(
        nc, in_maps, core_ids=list(range(NCORES)), trace=trace, **kw)
    total = res.results[0]["out"].astype(np.float64)
    for c in range(1, NCORES):
        total += res.results[c]["out"]
    total += np.asarray(inputs["bo"], np.float64)[None, :]
    out = total.astype(np.float32).reshape(1, S, D)
    return out, res


def kernel(**inputs):
    out, _ = run_full(inputs, trace=False)
    return out


# revision 13
# speedup vs baseline: 1.4062x; 1.1000x over previous
"""Trainium2 Bass kernel for ErnieImageAttention (non-causal MHA with per-head
RMSNorm on q/k + rotary embedding), tensor-parallel over heads on 8 NeuronCores.

Sharding: 16 heads / 8 cores = 2 heads per core. Each core computes its heads'
q/k/v projections, attention, and a partial output projection (row-parallel
Wo); the host sums the 8 partials and adds the bias.

Per-core dataflow (S=4096, D=2048, Hd=128, 2 local heads):
  phase 1 (per 128-row s-tile):
    q/k/v = hiddenT-chunk matmuls (bf16, N=256 = both heads) accumulated in
    PSUM; RMSNorm stats via Square-with-accum on ACT; RoPE via host-precomputed
    coefficient tables (g gains and the 1/sqrt(Hd) logit scale folded in; both
    r_q and r_k applied here); PE-transpose q/k 128x128 bf16 tiles into [d, s]
    layout; v kept [s, d] bf16.
  phase 2 (per 512-col q-block, per head, k in groups of 4 tiles):
    scoresT[k,q] = kT.T @ qT (bf16) into a 4-bank-wide PSUM region
    one wide Exp per group on ACT (amortizes the ~390ns ACTIVATE overhead)
    denominator: wide bf16 DVE adds, folded 2048->512, ones-matmul partition
    reduce, reciprocal_approx_fast, rank-1 ones matmul broadcast
    attn_T[d,q] = sum_k V[k,d]^T expT[k,q] accumulated in PSUM (bf16)
    normalize: attn_T * bcast(1/denom)
  phase 3 (inline per q-block): fin[s, :2048] = sum_h attnT_h.T @ WoT_h (f32r)

Softmax is max-subtraction-free: logits are ~N(0,1) by construction
(RMSNorm'd q/k, 1/sqrt(Hd) folded into q's rope tables).
"""

import numpy as np
import ml_dtypes

import concourse.bass as bass
import concourse.tile as tile
from concourse import bacc, mybir
from concourse import bass_utils
from concourse.masks import make_identity

F32 = mybir.dt.float32
F32R = mybir.dt.float32r
BF16 = mybir.dt.bfloat16
AX = mybir.AxisListType
AF = mybir.ActivationFunctionType

S = 4096
D = 2048
HD = 128
HEADS = 16
NCORES = 8
HLOC = HEADS // NCORES  # 2 heads per core
DLOC = HLOC * HD  # 256 local head dims
CH = D // 128  # 16 contraction chunks for projections
EPS = 1e-5
SCL = 1.0 / np.sqrt(HD)

QCOLS = 512  # q columns per attention block
KG = 4  # k tiles per wide-exp group


def build(nc, tc, io, s_len):
    st_n = s_len // 128  # s tiles
    qb_n = s_len // QCOLS  # q blocks
    qb_st = QCOLS // 128  # s tiles per q block
    kt_n = st_n  # k tiles
    kg_n = kt_n // KG  # k groups

    ht, wq, wk, wv, wo, cgq, sgq, cgk, sgk, out = (
        io["ht"], io["wq"], io["wk"], io["wv"], io["wo"],
        io["cgq"], io["sgq"], io["cgk"], io["sgk"], io["out"],
    )

    import contextlib

    with contextlib.ExitStack() as ctx:
        ctx.enter_context(nc.allow_low_precision(
            reason="bf16/f32r compute; values are O(1) and the rel-err "
                   "budget is 2e-2"))
        consts = ctx.enter_context(tc.tile_pool(name="consts", bufs=1))
        persist = ctx.enter_context(tc.tile_pool(name="persist", bufs=1))
        ht_pool = ctx.enter_context(tc.tile_pool(name="ht", bufs=2))
        cs_pool = ctx.enter_context(tc.tile_pool(name="cs", bufs=2))
        work = ctx.enter_context(tc.tile_pool(name="work", bufs=2))
        et_pool = ctx.enter_context(tc.tile_pool(name="et", bufs=3))
        at_pool = ctx.enter_context(tc.tile_pool(name="at", bufs=4))
        araw_pool = ctx.enter_context(tc.tile_pool(name="araw", bufs=2))
        acc_pool = ctx.enter_context(tc.tile_pool(name="acc", bufs=2))
        rc_pool = ctx.enter_context(tc.tile_pool(name="rc", bufs=1))
        fin_pool = ctx.enter_context(tc.tile_pool(name="fin", bufs=4))

        # constants
        ident = consts.tile([128, 128], BF16)
        make_identity(nc, ident[:])
        ones_f32 = consts.tile([128, 1], F32)
        nc.vector.memset(ones_f32[:], 1.0)
        ones_col = consts.tile([128, 1], F32R)
        nc.vector.tensor_copy(ones_col[:], ones_f32[:])
        ones_col_bf = consts.tile([128, 1], BF16)
        nc.vector.tensor_copy(ones_col_bf[:], ones_f32[:])
        ones_row_f32 = consts.tile([1, 128], F32)
        nc.vector.memset(ones_row_f32[:], 1.0)
        ones_row = consts.tile([1, 128], F32R)
        nc.vector.tensor_copy(ones_row[:], ones_row_f32[:])
        eps_t = consts.tile([128, 1], F32)
        nc.vector.memset(eps_t[:], EPS)

        wq_sb = consts.tile([128, CH, DLOC], BF16)
        nc.sync.dma_start(out=wq_sb[:], in_=wq)
        wk_sb = consts.tile([128, CH, DLOC], BF16)
        nc.sync.dma_start(out=wk_sb[:], in_=wk)
        wv_sb = consts.tile([128, CH, DLOC], BF16)
        nc.sync.dma_start(out=wv_sb[:], in_=wv)
        wo_sb = consts.tile([128, HLOC, D], F32R)
        nc.sync.dma_start(out=wo_sb[:], in_=wo)

        # persistent per-head transposed q/k and v
        qT_sb = persist.tile([128, HLOC, st_n, 128], BF16)
        kT_sb = persist.tile([128, HLOC, st_n, 128], BF16)
        v_sb = persist.tile([128, st_n, DLOC], BF16)

        # ---------------- phase 1: projections + norm + rope + transpose ----
        with tc.tile_pool(name="ps1", bufs=2, space="PSUM") as ps1:
            for st in range(st_n):
                ss = slice(st * 128, (st + 1) * 128)
                ht_t = ht_pool.tile([128, CH, 128], BF16, tag="ht")
                nc.sync.dma_start(out=ht_t[:], in_=ht[st])
                cgq_t = cs_pool.tile([128, DLOC], F32, tag="cgq")
                nc.sync.dma_start(out=cgq_t[:], in_=cgq[ss, :])
                sgq_t = cs_pool.tile([128, DLOC], F32, tag="sgq")
                nc.sync.dma_start(out=sgq_t[:], in_=sgq[ss, :])
                cgk_t = cs_pool.tile([128, DLOC], F32, tag="cgk")
                nc.sync.dma_start(out=cgk_t[:], in_=cgk[ss, :])
                sgk_t = cs_pool.tile([128, DLOC], F32, tag="sgk")
                nc.sync.dma_start(out=sgk_t[:], in_=sgk[ss, :])

                pq = ps1.tile([128, DLOC], F32, tag="pq")
                pk = ps1.tile([128, DLOC], F32, tag="pk")
                pv = ps1.tile([128, DLOC], F32, tag="pv")
                for c in range(CH):
                    lhs = ht_t[:, c, :]
                    nc.tensor.matmul(pq[:], lhs, wq_sb[:, c, :],
                                     start=(c == 0), stop=(c == CH - 1))
                    nc.tensor.matmul(pk[:], lhs, wk_sb[:, c, :],
                                     start=(c == 0), stop=(c == CH - 1))
                    nc.tensor.matmul(pv[:], lhs, wv_sb[:, c, :],
                                     start=(c == 0), stop=(c == CH - 1))

                # v: PSUM -> SBUF bf16
                nc.scalar.copy(v_sb[:, st, :], pv[:])

                # rms stats: Square with free-dim accumulation -> sum(q^2)
                varq = work.tile([128, HLOC], F32, tag="varq")
                vark = work.tile([128, HLOC], F32, tag="vark")
                sqd = work.tile([128, HD], F32, tag="sqd")  # dump
                for h in range(HLOC):
                    hs = slice(h * HD, (h + 1) * HD)
                    nc.scalar.activation(sqd[:], pq[:, hs], AF.Square,
                                         accum_out=varq[:, h:h + 1])
                    nc.scalar.activation(sqd[:], pk[:, hs], AF.Square,
                                         accum_out=vark[:, h:h + 1])
                sigq = work.tile([128, HLOC], F32, tag="sigq")
                nc.scalar.activation(sigq[:], varq[:], AF.Sqrt,
                                     bias=eps_t[:], scale=1.0 / HD)
                rq = work.tile([128, HLOC], F32, tag="rq")
                nc.vector.reciprocal_approx_fast(rq[:], sigq[:])
                sigk = work.tile([128, HLOC], F32, tag="sigk")
                nc.scalar.activation(sigk[:], vark[:], AF.Sqrt,
                                     bias=eps_t[:], scale=1.0 / HD)
                rk = work.tile([128, HLOC], F32, tag="rk")
                nc.vector.reciprocal_approx_fast(rk[:], sigk[:])

                # rope: out = (r*x) . CG + shift64(r*x) . SG   (per tensor)
                for name, psrc, r, cg, sg, dstT in (
                    ("q", pq, rq, cgq_t, sgq_t, qT_sb),
                    ("k", pk, rk, cgk_t, sgk_t, kT_sb),
                ):
                    xs = work.tile([128, DLOC], F32, tag=f"xs{name}")
                    for h in range(HLOC):
                        hs = slice(h * HD, (h + 1) * HD)
                        nc.vector.tensor_scalar_mul(xs[:, hs], psrc[:, hs],
                                                    r[:, h:h + 1])
                    m1 = work.tile([128, DLOC], F32, tag=f"m1{name}")
                    nc.vector.tensor_mul(m1[:], xs[:], cg[:])
                    m2 = work.tile([128, DLOC], F32, tag=f"m2{name}")
                    x4 = xs[:].rearrange("p (h t u) -> p h t u", h=HLOC, t=2)
                    m4 = m2[:].rearrange("p (h t u) -> p h t u", h=HLOC, t=2)
                    g4 = sg[:].rearrange("p (h t u) -> p h t u", h=HLOC, t=2)
                    nc.vector.tensor_mul(m4[:, :, 0, :], x4[:, :, 1, :],
                                         g4[:, :, 0, :])
                    nc.vector.tensor_mul(m4[:, :, 1, :], x4[:, :, 0, :],
                                         g4[:, :, 1, :])
                    xa = work.tile([128, DLOC], BF16, tag=f"xa{name}")
                    nc.vector.tensor_add(xa[:], m1[:], m2[:])
                    for h in range(HLOC):
                        hs = slice(h * HD, (h + 1) * HD)
                        ptp = ps1.tile([128, 128], BF16, tag="ptp")
                        nc.tensor.transpose(ptp[:], xa[:, hs], ident[:])
                        nc.scalar.copy(dstT[:, h, st, :], ptp[:])

        # ---------------- phase 2+3: attention + output projection ----------
        # Ping-pong wide score buffers (4-bank A / 2-bank B) with software
        # pipelining: group g+1's score matmuls are emitted BEFORE group g's
        # PV matmuls so the in-order PE queue never stalls behind ACT's exp.
        # The previous q-block's output projection is drip-fed between groups
        # so it overlaps the current k-loop instead of serializing after it.
        with (
            tc.tile_pool(name="psA", bufs=1, space="PSUM") as psA,
            tc.tile_pool(name="psB", bufs=2, space="PSUM") as psB,
        ):
            groups = []  # (kt0, glen, tag)
            kt0 = 0
            gi = 0
            while kt0 < kt_n:
                tag = "scA" if gi % 2 == 0 else "scB"
                cap = 4 if tag == "scA" else 2
                glen = min(cap, kt_n - kt0)
                groups.append((kt0, glen, tag))
                kt0 += glen
                gi += 1
            lanesA = max((g[1] for g in groups if g[2] == "scA"), default=0)
            lanesB = max((g[1] for g in groups if g[2] == "scB"), default=0)

            pending = []  # out-proj chunk emitters from the previous q block

            def outproj_chunks(qb, ats):
                chunks = []
                for sti in range(qb_st):
                    st = qb * qb_st + sti
                    sl = slice(sti * 128, (sti + 1) * 128)
                    for nchunk in range(D // 512):
                        ns = slice(nchunk * 512, (nchunk + 1) * 512)

                        def emit(st=st, sl=sl, ns=ns, ats=ats):
                            pf = psB.tile([128, QCOLS], F32, tag="po",
                                          name=f"pf_{st}_{ns.start}")
                            for h in range(HLOC):
                                nc.tensor.matmul(pf[:], ats[h][:, sl],
                                                 wo_sb[:, h, ns],
                                                 start=(h == 0),
                                                 stop=(h == HLOC - 1))
                            fin = fin_pool.tile([128, 512], F32R, tag="fin")
                            nc.vector.tensor_copy(fin[:], pf[:])
                            nc.sync.dma_start(
                                out=out[st * 128:(st + 1) * 128, ns],
                                in_=fin[:])
                        chunks.append(emit)
                return chunks

            for qb in range(qb_n):
                ats = []
                for h in range(HLOC):
                    accA = acc_pool.tile([128, 4, QCOLS], BF16, tag="accA")
                    accB = None
                    if lanesB:
                        accB = acc_pool.tile([128, 2, QCOLS], BF16,
                                             tag="accB", name="accB")
                    first = {"scA": True, "scB": True}
                    po = psB.tile([128, QCOLS], F32, tag="po")
                    q_rhs = qT_sb[:, h, qb * qb_st:(qb + 1) * qb_st, :]

                    def flush(et, kt0, glen, tag):
                        acc = accA if tag == "scA" else accB
                        if first[tag]:
                            nc.vector.tensor_copy(acc[:, 0:glen, :],
                                                  et[:, 0:glen, :])
                            first[tag] = False
                        else:
                            nc.vector.tensor_add(acc[:, 0:glen, :],
                                                 acc[:, 0:glen, :],
                                                 et[:, 0:glen, :])
                        for j in range(glen):
                            kt = kt0 + j
                            nc.tensor.matmul(po[:],
                                             v_sb[:, kt, h * HD:(h + 1) * HD],
                                             et[:, j, :], start=(kt == 0),
                                             stop=(kt == kt_n - 1))

                    prev = None
                    for (kt0, glen, tag) in groups:
                        width = 4 if tag == "scA" else 2
                        sc = psA.tile([128, width, QCOLS], F32, tag=tag,
                                      name=f"sc_{tag}")
                        for j in range(glen):
                            nc.tensor.matmul(sc[:, j, :],
                                             kT_sb[:, h, kt0 + j, :],
                                             q_rhs, start=True, stop=True)
                        et = et_pool.tile([128, 4, QCOLS], BF16, tag="et")
                        nc.scalar.activation(et[:, 0:glen, :],
                                             sc[:, 0:glen, :], AF.Exp)
                        if prev is not None:
                            flush(*prev)
                        prev = (et, kt0, glen, tag)
                        if pending:
                            pending.pop(0)()
                    flush(*prev)

                    # attn normalization tail: araw early (only needs po),
                    # lane ones-matmuls accumulate the denominator in PSUM,
                    # 1/x, rank-1 broadcast (pd/pb live in the scB slot)
                    araw = araw_pool.tile([128, QCOLS], F32R, tag="araw")
                    nc.scalar.copy(araw[:], po[:])
                    pdb = psA.tile([128, 2, QCOLS], F32, tag="scB",
                                   name="pdb")
                    pd = pdb[0:1, 0, :]
                    lanes = []
                    if accB is not None:
                        lanes += [accB[:, j, :] for j in range(lanesB)]
                    lanes += [accA[:, j, :] for j in range(lanesA)]
                    for i, lane in enumerate(lanes):
                        nc.tensor.matmul(pd, ones_col_bf[:], lane,
                                         start=(i == 0),
                                         stop=(i == len(lanes) - 1))
                    rsb = rc_pool.tile([1, QCOLS], F32, tag="rsb")
                    nc.vector.reciprocal_approx_fast(rsb[:], pd)
                    rsr = rc_pool.tile([1, QCOLS], F32R, tag="rsr")
                    nc.vector.tensor_copy(rsr[:], rsb[:])
                    nc.tensor.matmul(pdb[:, 0, :], ones_row[:], rsr[:],
                                     start=True, stop=True)
                    at = at_pool.tile([128, QCOLS], F32R, tag="at")
                    nc.vector.tensor_mul(at[:], araw[:], pdb[:, 0, :])
                    ats.append(at)

                while pending:
                    pending.pop(0)()
                pending = outproj_chunks(qb, ats)
            while pending:
                pending.pop(0)()


def build_program(s_len=S):
    nc = bacc.Bacc("TRN2", target_bir_lowering=False, debug=False,
                   enable_asserts=False)
    st_n = s_len // 128
    io = {
        "ht": nc.dram_tensor("ht", [st_n, 128, CH, 128], BF16,
                             kind="ExternalInput").ap(),
        "wq": nc.dram_tensor("wq", [128, CH, DLOC], BF16,
                             kind="ExternalInput").ap(),
        "wk": nc.dram_tensor("wk", [128, CH, DLOC], BF16,
                             kind="ExternalInput").ap(),
        "wv": nc.dram_tensor("wv", [128, CH, DLOC], BF16,
                             kind="ExternalInput").ap(),
        "wo": nc.dram_tensor("wo", [128, HLOC, D], F32R,
                             kind="ExternalInput").ap(),
        "cgq": nc.dram_tensor("cgq", [s_len, DLOC], F32,
                              kind="ExternalInput").ap(),
        "sgq": nc.dram_tensor("sgq", [s_len, DLOC], F32,
                              kind="ExternalInput").ap(),
        "cgk": nc.dram_tensor("cgk", [s_len, DLOC], F32,
                              kind="ExternalInput").ap(),
        "sgk": nc.dram_tensor("sgk", [s_len, DLOC], F32,
                              kind="ExternalInput").ap(),
        "out": nc.dram_tensor("out", [s_len, D], F32R,
                              kind="ExternalOutput").ap(),
    }
    with tile.TileContext(nc) as tc:
        build(nc, tc, io, s_len)
    nc.compile()
    return nc


def prep_inputs(inputs, s_len=S):
    """Host-side preprocessing: transposed/tiled bf16 layouts + rope
    coefficient tables (g gains and the 1/sqrt(Hd) scale folded in,
    duplicated per local head for full-width elementwise ops)."""
    bf16 = ml_dtypes.bfloat16
    hs = np.asarray(inputs["hidden_states"], np.float32).reshape(s_len, D)
    st_n = s_len // 128
    ht = np.ascontiguousarray(
        hs.reshape(st_n, 128, CH, 128).transpose(0, 3, 2, 1)).astype(bf16)

    fc = np.asarray(inputs["freqs_cis"], np.float32).reshape(s_len, HD)
    cos = np.cos(fc)
    sin = np.sin(fc)
    gq = np.asarray(inputs["gq"], np.float32)
    gk = np.asarray(inputs["gk"], np.float32)

    def coef(g, scale):
        cg = cos * g[None, :] * scale
        sg = np.empty_like(sin)
        sg[:, :64] = -sin[:, :64] * g[None, 64:] * scale
        sg[:, 64:] = sin[:, 64:] * g[None, :64] * scale
        cg2 = np.ascontiguousarray(np.tile(cg, (1, HLOC)))
        sg2 = np.ascontiguousarray(np.tile(sg, (1, HLOC)))
        return cg2, sg2

    cgq, sgq = coef(gq, SCL)
    cgk, sgk = coef(gk, 1.0)

    Wq = np.asarray(inputs["Wq"], np.float32)
    Wk = np.asarray(inputs["Wk"], np.float32)
    Wv = np.asarray(inputs["Wv"], np.float32)
    Wo = np.asarray(inputs["Wo"], np.float32)

    in_maps = []
    for c in range(NCORES):
        cols = slice(DLOC * c, DLOC * (c + 1))
        wq_c = np.ascontiguousarray(
            Wq[cols, :].T.reshape(CH, 128, DLOC).transpose(1, 0, 2)).astype(bf16)
        wk_c = np.ascontiguousarray(
            Wk[cols, :].T.reshape(CH, 128, DLOC).transpose(1, 0, 2)).astype(bf16)
        wv_c = np.ascontiguousarray(
            Wv[cols, :].T.reshape(CH, 128, DLOC).transpose(1, 0, 2)).astype(bf16)
        wo_c = np.ascontiguousarray(
            Wo[:, cols].T.reshape(HLOC, 128, D).transpose(1, 0, 2))
        in_maps.append({
            "ht": ht, "wq": wq_c, "wk": wk_c, "wv": wv_c, "wo": wo_c,
            "cgq": cgq, "sgq": sgq, "cgk": cgk, "sgk": sgk,
        })
    return in_maps


_CACHE = {}


def run_full(inputs, trace=False, **kw):
    if "nc" not in _CACHE:
        _CACHE["nc"] = build_program(S)
    nc = _CACHE["nc"]
    in_maps = prep_inputs(inputs, S)
    res = bass_utils.run_bass_kernel_spmd(
        nc, in_maps, core_ids=list(range(NCORES)), trace=trace, **kw)
    total = res.results[0]["out"].astype(np.float64)
    for c in range(1, NCORES):
        total += res.results[c]["out"]
    total += np.asarray(inputs["bo"], np.float64)[None, :]
    out = total.astype(np.float32).reshape(1, S, D)
    return out, res


def kernel(**inputs):
    out, _ = run_full(inputs, trace=False)
    return out


# revision 14
# speedup vs baseline: 1.4581x; 1.0370x over previous
"""Trainium2 Bass kernel for ErnieImageAttention (non-causal MHA with per-head
RMSNorm on q/k + rotary embedding), tensor-parallel over heads on 8 NeuronCores.

Sharding: 16 heads / 8 cores = 2 heads per core. Each core computes its heads'
q/k/v projections, attention, and a partial output projection (row-parallel
Wo); the host sums the 8 partials and adds the bias.

Per-core dataflow (S=4096, D=2048, Hd=128, 2 local heads):
  phase 1 (per 128-row s-tile):
    q/k/v = hiddenT-chunk matmuls (bf16, N=256 = both heads) accumulated in
    PSUM; RMSNorm stats via Square-with-accum on ACT; RoPE via host-precomputed
    coefficient tables (g gains and the 1/sqrt(Hd) logit scale folded in; both
    r_q and r_k applied here); PE-transpose q/k 128x128 bf16 tiles into [d, s]
    layout; v kept [s, d] bf16.
  phase 2 (per 512-col q-block, per head, k in groups of 4 tiles):
    scoresT[k,q] = kT.T @ qT (bf16) into a 4-bank-wide PSUM region
    one wide Exp per group on ACT (amortizes the ~390ns ACTIVATE overhead)
    denominator: wide bf16 DVE adds, folded 2048->512, ones-matmul partition
    reduce, reciprocal_approx_fast, rank-1 ones matmul broadcast
    attn_T[d,q] = sum_k V[k,d]^T expT[k,q] accumulated in PSUM (bf16)
    normalize: attn_T * bcast(1/denom)
  phase 3 (inline per q-block): fin[s, :2048] = sum_h attnT_h.T @ WoT_h (f32r)

Softmax is max-subtraction-free: logits are ~N(0,1) by construction
(RMSNorm'd q/k, 1/sqrt(Hd) folded into q's rope tables).
"""

import numpy as np
import ml_dtypes

import concourse.bass as bass
import concourse.tile as tile
from concourse import bacc, mybir
from concourse import bass_utils
from concourse.masks import make_identity

F32 = mybir.dt.float32
F32R = mybir.dt.float32r
BF16 = mybir.dt.bfloat16
AX = mybir.AxisListType
AF = mybir.ActivationFunctionType

S = 4096
D = 2048
HD = 128
HEADS = 16
NCORES = 8
HLOC = HEADS // NCORES  # 2 heads per core
DLOC = HLOC * HD  # 256 local head dims
CH = D // 128  # 16 contraction chunks for projections
EPS = 1e-5
SCL = 1.0 / np.sqrt(HD)

QCOLS = 512  # q columns per attention block
KG = 4  # k tiles per wide-exp group


def build(nc, tc, io, s_len):
    st_n = s_len // 128  # s tiles
    qb_n = s_len // QCOLS  # q blocks
    qb_st = QCOLS // 128  # s tiles per q block
    kt_n = st_n  # k tiles
    kg_n = kt_n // KG  # k groups

    ht, wq, wk, wv, wo, cgq, sgq, cgk, sgk, out = (
        io["ht"], io["wq"], io["wk"], io["wv"], io["wo"],
        io["cgq"], io["sgq"], io["cgk"], io["sgk"], io["out"],
    )

    import contextlib

    with contextlib.ExitStack() as ctx:
        ctx.enter_context(nc.allow_low_precision(
            reason="bf16/f32r compute; values are O(1) and the rel-err "
                   "budget is 2e-2"))
        consts = ctx.enter_context(tc.tile_pool(name="consts", bufs=1))
        persist = ctx.enter_context(tc.tile_pool(name="persist", bufs=1))
        ht_pool = ctx.enter_context(tc.tile_pool(name="ht", bufs=2))
        cs_pool = ctx.enter_context(tc.tile_pool(name="cs", bufs=2))
        work = ctx.enter_context(tc.tile_pool(name="work", bufs=2))
        et_pool = ctx.enter_context(tc.tile_pool(name="et", bufs=3))
        at_pool = ctx.enter_context(tc.tile_pool(name="at", bufs=4))
        araw_pool = ctx.enter_context(tc.tile_pool(name="araw", bufs=2))
        acc_pool = ctx.enter_context(tc.tile_pool(name="acc", bufs=2))
        rc_pool = ctx.enter_context(tc.tile_pool(name="rc", bufs=1))
        fin_pool = ctx.enter_context(tc.tile_pool(name="fin", bufs=4))

        # constants
        ident = consts.tile([128, 128], BF16)
        make_identity(nc, ident[:])
        ones_f32 = consts.tile([128, 1], F32)
        nc.vector.memset(ones_f32[:], 1.0)
        ones_col = consts.tile([128, 1], F32R)
        nc.vector.tensor_copy(ones_col[:], ones_f32[:])
        ones_col_bf = consts.tile([128, 1], BF16)
        nc.vector.tensor_copy(ones_col_bf[:], ones_f32[:])
        ones_row_f32 = consts.tile([1, 128], F32)
        nc.vector.memset(ones_row_f32[:], 1.0)
        ones_row = consts.tile([1, 128], F32R)
        nc.vector.tensor_copy(ones_row[:], ones_row_f32[:])
        eps_t = consts.tile([128, 1], F32)
        nc.vector.memset(eps_t[:], EPS)

        wq_sb = consts.tile([128, CH, DLOC], BF16)
        nc.sync.dma_start(out=wq_sb[:], in_=wq)
        wk_sb = consts.tile([128, CH, DLOC], BF16)
        nc.sync.dma_start(out=wk_sb[:], in_=wk)
        wv_sb = consts.tile([128, CH, DLOC], BF16)
        nc.sync.dma_start(out=wv_sb[:], in_=wv)
        wo_sb = consts.tile([128, HLOC, D], F32R)
        nc.sync.dma_start(out=wo_sb[:], in_=wo)

        # persistent per-head transposed q/k and v
        qT_sb = persist.tile([128, HLOC, st_n, 128], BF16)
        kT_sb = persist.tile([128, HLOC, st_n, 128], BF16)
        v_sb = persist.tile([128, st_n, DLOC], BF16)

        # ---------------- phase 1: projections + norm + rope + transpose ----
        with tc.tile_pool(name="ps1", bufs=2, space="PSUM") as ps1:
            for st in range(st_n):
                ss = slice(st * 128, (st + 1) * 128)
                ht_t = ht_pool.tile([128, CH, 128], BF16, tag="ht")
                nc.sync.dma_start(out=ht_t[:], in_=ht[st])
                cgq_t = cs_pool.tile([128, DLOC], F32, tag="cgq")
                nc.sync.dma_start(out=cgq_t[:], in_=cgq[ss, :])
                sgq_t = cs_pool.tile([128, DLOC], F32, tag="sgq")
                nc.sync.dma_start(out=sgq_t[:], in_=sgq[ss, :])
                cgk_t = cs_pool.tile([128, DLOC], F32, tag="cgk")
                nc.sync.dma_start(out=cgk_t[:], in_=cgk[ss, :])
                sgk_t = cs_pool.tile([128, DLOC], F32, tag="sgk")
                nc.sync.dma_start(out=sgk_t[:], in_=sgk[ss, :])

                pq = ps1.tile([128, DLOC], F32, tag="pq")
                pk = ps1.tile([128, DLOC], F32, tag="pk")
                pv = ps1.tile([128, DLOC], F32, tag="pv")
                for c in range(CH):
                    lhs = ht_t[:, c, :]
                    nc.tensor.matmul(pq[:], lhs, wq_sb[:, c, :],
                                     start=(c == 0), stop=(c == CH - 1))
                    nc.tensor.matmul(pk[:], lhs, wk_sb[:, c, :],
                                     start=(c == 0), stop=(c == CH - 1))
                    nc.tensor.matmul(pv[:], lhs, wv_sb[:, c, :],
                                     start=(c == 0), stop=(c == CH - 1))

                # v: PSUM -> SBUF bf16
                nc.scalar.copy(v_sb[:, st, :], pv[:])

                # rms stats: Square with free-dim accumulation -> sum(q^2)
                varq = work.tile([128, HLOC], F32, tag="varq")
                vark = work.tile([128, HLOC], F32, tag="vark")
                sqd = work.tile([128, HD], F32, tag="sqd")  # dump
                for h in range(HLOC):
                    hs = slice(h * HD, (h + 1) * HD)
                    nc.scalar.activation(sqd[:], pq[:, hs], AF.Square,
                                         accum_out=varq[:, h:h + 1])
                    nc.scalar.activation(sqd[:], pk[:, hs], AF.Square,
                                         accum_out=vark[:, h:h + 1])
                sigq = work.tile([128, HLOC], F32, tag="sigq")
                nc.scalar.activation(sigq[:], varq[:], AF.Sqrt,
                                     bias=eps_t[:], scale=1.0 / HD)
                rq = work.tile([128, HLOC], F32, tag="rq")
                nc.vector.reciprocal_approx_fast(rq[:], sigq[:])
                sigk = work.tile([128, HLOC], F32, tag="sigk")
                nc.scalar.activation(sigk[:], vark[:], AF.Sqrt,
                                     bias=eps_t[:], scale=1.0 / HD)
                rk = work.tile([128, HLOC], F32, tag="rk")
                nc.vector.reciprocal_approx_fast(rk[:], sigk[:])

                # rope: out = (r*x) . CG + shift64(r*x) . SG   (per tensor)
                for name, psrc, r, cg, sg, dstT in (
                    ("q", pq, rq, cgq_t, sgq_t, qT_sb),
                    ("k", pk, rk, cgk_t, sgk_t, kT_sb),
                ):
                    xs = work.tile([128, DLOC], F32, tag=f"xs{name}")
                    for h in range(HLOC):
                        hs = slice(h * HD, (h + 1) * HD)
                        nc.vector.tensor_scalar_mul(xs[:, hs], psrc[:, hs],
                                                    r[:, h:h + 1])
                    m1 = work.tile([128, DLOC], F32, tag=f"m1{name}")
                    nc.vector.tensor_mul(m1[:], xs[:], cg[:])
                    m2 = work.tile([128, DLOC], F32, tag=f"m2{name}")
                    x4 = xs[:].rearrange("p (h t u) -> p h t u", h=HLOC, t=2)
                    m4 = m2[:].rearrange("p (h t u) -> p h t u", h=HLOC, t=2)
                    g4 = sg[:].rearrange("p (h t u) -> p h t u", h=HLOC, t=2)
                    nc.vector.tensor_mul(m4[:, :, 0, :], x4[:, :, 1, :],
                                         g4[:, :, 0, :])
                    nc.vector.tensor_mul(m4[:, :, 1, :], x4[:, :, 0, :],
                                         g4[:, :, 1, :])
                    xa = work.tile([128, DLOC], BF16, tag=f"xa{name}")
                    nc.vector.tensor_add(xa[:], m1[:], m2[:])
                    for h in range(HLOC):
                        hs = slice(h * HD, (h + 1) * HD)
                        ptp = ps1.tile([128, 128], BF16, tag="ptp")
                        nc.tensor.transpose(ptp[:], xa[:, hs], ident[:])
                        nc.scalar.copy(dstT[:, h, st, :], ptp[:])

        # ---------------- phase 2+3: attention + output projection ----------
        # Ping-pong wide score buffers (4-bank A / 2-bank B) with software
        # pipelining: group g+1's score matmuls are emitted BEFORE group g's
        # PV matmuls so the in-order PE queue never stalls behind ACT's exp.
        # The previous q-block's output projection is drip-fed between groups
        # so it overlaps the current k-loop instead of serializing after it.
        with (
            tc.tile_pool(name="psA", bufs=1, space="PSUM") as psA,
            tc.tile_pool(name="psB", bufs=2, space="PSUM") as psB,
        ):
            groups = []  # (kt0, glen, tag)
            kt0 = 0
            gi = 0
            while kt0 < kt_n:
                tag = "scA" if gi % 2 == 0 else "scB"
                cap = 4 if tag == "scA" else 2
                glen = min(cap, kt_n - kt0)
                groups.append((kt0, glen, tag))
                kt0 += glen
                gi += 1
            lanesA = max((g[1] for g in groups if g[2] == "scA"), default=0)
            lanesB = max((g[1] for g in groups if g[2] == "scB"), default=0)

            pending = []  # out-proj chunk emitters from the previous q block

            def outproj_chunks(qb, ats):
                chunks = []
                for sti in range(qb_st):
                    st = qb * qb_st + sti
                    sl = slice(sti * 128, (sti + 1) * 128)
                    for nchunk in range(D // 512):
                        ns = slice(nchunk * 512, (nchunk + 1) * 512)

                        def emit(st=st, sl=sl, ns=ns, ats=ats):
                            pf = psB.tile([128, QCOLS], F32, tag="po",
                                          name=f"pf_{st}_{ns.start}")
                            for h in range(HLOC):
                                nc.tensor.matmul(pf[:], ats[h][:, sl],
                                                 wo_sb[:, h, ns],
                                                 start=(h == 0),
                                                 stop=(h == HLOC - 1))
                            fin = fin_pool.tile([128, 512], F32R, tag="fin")
                            nc.vector.tensor_copy(fin[:], pf[:])
                            nc.sync.dma_start(
                                out=out[st * 128:(st + 1) * 128, ns],
                                in_=fin[:])
                        chunks.append(emit)
                return chunks

            for qb in range(qb_n):
                ats = []
                for h in range(HLOC):
                    accA = acc_pool.tile([128, 4, QCOLS], BF16, tag="accA")
                    accB = None
                    if lanesB:
                        accB = acc_pool.tile([128, 2, QCOLS], BF16,
                                             tag="accB", name="accB")
                    first = {"scA": True, "scB": True}
                    po = psB.tile([128, QCOLS], F32, tag="po")
                    q_rhs = qT_sb[:, h, qb * qb_st:(qb + 1) * qb_st, :]

                    def flush(et, kt0, glen, tag):
                        acc = accA if tag == "scA" else accB
                        if first[tag]:
                            nc.vector.tensor_copy(acc[:, 0:glen, :],
                                                  et[:, 0:glen, :])
                            first[tag] = False
                        else:
                            nc.vector.tensor_add(acc[:, 0:glen, :],
                                                 acc[:, 0:glen, :],
                                                 et[:, 0:glen, :])
                        for j in range(glen):
                            kt = kt0 + j
                            nc.tensor.matmul(po[:],
                                             v_sb[:, kt, h * HD:(h + 1) * HD],
                                             et[:, j, :], start=(kt == 0),
                                             stop=(kt == kt_n - 1))

                    prev = None
                    for (kt0, glen, tag) in groups:
                        width = 4 if tag == "scA" else 2
                        sc = psA.tile([128, width, QCOLS], F32, tag=tag,
                                      name=f"sc_{tag}")
                        for j in range(glen):
                            nc.tensor.matmul(sc[:, j, :],
                                             kT_sb[:, h, kt0 + j, :],
                                             q_rhs, start=True, stop=True)
                        et = et_pool.tile([128, 4, QCOLS], BF16, tag="et")
                        nc.scalar.activation(et[:, 0:glen, :],
                                             sc[:, 0:glen, :], AF.Exp)
                        if prev is not None:
                            flush(*prev)
                        prev = (et, kt0, glen, tag)
                        if pending:
                            pending.pop(0)()
                    flush(*prev)

                    # attn normalization tail: araw emitted now (only needs
                    # po); the PE pieces (lane ones-matmuls, broadcast) are
                    # deferred into the next head/q-block's k-loop so they
                    # don't block the in-order PE queue.
                    araw = araw_pool.tile([128, QCOLS], F32R, tag="araw")
                    nc.scalar.copy(araw[:], po[:])
                    at = at_pool.tile([128, QCOLS], F32R, tag="at")
                    ats.append(at)

                    def tail(accA=accA, accB=accB, araw=araw, at=at):
                        pdb = psA.tile([128, 2, QCOLS], F32, tag="scB",
                                       name="pdb")
                        pd = pdb[0:1, 0, :]
                        lanes = []
                        if accB is not None:
                            lanes += [accB[:, j, :] for j in range(lanesB)]
                        lanes += [accA[:, j, :] for j in range(lanesA)]
                        for i, lane in enumerate(lanes):
                            nc.tensor.matmul(pd, ones_col_bf[:], lane,
                                             start=(i == 0),
                                             stop=(i == len(lanes) - 1))
                        rsb = rc_pool.tile([1, QCOLS], F32, tag="rsb")
                        nc.vector.reciprocal_approx_fast(rsb[:], pd)
                        rsr = rc_pool.tile([1, QCOLS], F32R, tag="rsr")
                        nc.vector.tensor_copy(rsr[:], rsb[:])
                        nc.tensor.matmul(pdb[:, 0, :], ones_row[:], rsr[:],
                                         start=True, stop=True)
                        nc.vector.tensor_mul(at[:], araw[:], pdb[:, 0, :])
                    pending.insert(0, tail)

                while len(pending) > 2:
                    pending.pop(0)()
                pending += outproj_chunks(qb, ats)
            while pending:
                pending.pop(0)()


def build_program(s_len=S):
    nc = bacc.Bacc("TRN2", target_bir_lowering=False, debug=False,
                   enable_asserts=False)
    st_n = s_len // 128
    io = {
        "ht": nc.dram_tensor("ht", [st_n, 128, CH, 128], BF16,
                             kind="ExternalInput").ap(),
        "wq": nc.dram_tensor("wq", [128, CH, DLOC], BF16,
                             kind="ExternalInput").ap(),
        "wk": nc.dram_tensor("wk", [128, CH, DLOC], BF16,
                             kind="ExternalInput").ap(),
        "wv": nc.dram_tensor("wv", [128, CH, DLOC], BF16,
                             kind="ExternalInput").ap(),
        "wo": nc.dram_tensor("wo", [128, HLOC, D], F32R,
                             kind="ExternalInput").ap(),
        "cgq": nc.dram_tensor("cgq", [s_len, DLOC], F32,
                              kind="ExternalInput").ap(),
        "sgq": nc.dram_tensor("sgq", [s_len, DLOC], F32,
                              kind="ExternalInput").ap(),
        "cgk": nc.dram_tensor("cgk", [s_len, DLOC], F32,
                              kind="ExternalInput").ap(),
        "sgk": nc.dram_tensor("sgk", [s_len, DLOC], F32,
                              kind="ExternalInput").ap(),
        "out": nc.dram_tensor("out", [s_len, D], F32R,
                              kind="ExternalOutput").ap(),
    }
    with tile.TileContext(nc) as tc:
        build(nc, tc, io, s_len)
    nc.compile()
    return nc


def prep_inputs(inputs, s_len=S):
    """Host-side preprocessing: transposed/tiled bf16 layouts + rope
    coefficient tables (g gains and the 1/sqrt(Hd) scale folded in,
    duplicated per local head for full-width elementwise ops)."""
    bf16 = ml_dtypes.bfloat16
    hs = np.asarray(inputs["hidden_states"], np.float32).reshape(s_len, D)
    st_n = s_len // 128
    ht = np.ascontiguousarray(
        hs.reshape(st_n, 128, CH, 128).transpose(0, 3, 2, 1)).astype(bf16)

    fc = np.asarray(inputs["freqs_cis"], np.float32).reshape(s_len, HD)
    cos = np.cos(fc)
    sin = np.sin(fc)
    gq = np.asarray(inputs["gq"], np.float32)
    gk = np.asarray(inputs["gk"], np.float32)

    def coef(g, scale):
        cg = cos * g[None, :] * scale
        sg = np.empty_like(sin)
        sg[:, :64] = -sin[:, :64] * g[None, 64:] * scale
        sg[:, 64:] = sin[:, 64:] * g[None, :64] * scale
        cg2 = np.ascontiguousarray(np.tile(cg, (1, HLOC)))
        sg2 = np.ascontiguousarray(np.tile(sg, (1, HLOC)))
        return cg2, sg2

    cgq, sgq = coef(gq, SCL)
    cgk, sgk = coef(gk, 1.0)

    Wq = np.asarray(inputs["Wq"], np.float32)
    Wk = np.asarray(inputs["Wk"], np.float32)
    Wv = np.asarray(inputs["Wv"], np.float32)
    Wo = np.asarray(inputs["Wo"], np.float32)

    in_maps = []
    for c in range(NCORES):
        cols = slice(DLOC * c, DLOC * (c + 1))
        wq_c = np.ascontiguousarray(
            Wq[cols, :].T.reshape(CH, 128, DLOC).transpose(1, 0, 2)).astype(bf16)
        wk_c = np.ascontiguousarray(
            Wk[cols, :].T.reshape(CH, 128, DLOC).transpose(1, 0, 2)).astype(bf16)
        wv_c = np.ascontiguousarray(
            Wv[cols, :].T.reshape(CH, 128, DLOC).transpose(1, 0, 2)).astype(bf16)
        wo_c = np.ascontiguousarray(
            Wo[:, cols].T.reshape(HLOC, 128, D).transpose(1, 0, 2))
        in_maps.append({
            "ht": ht, "wq": wq_c, "wk": wk_c, "wv": wv_c, "wo": wo_c,
            "cgq": cgq, "sgq": sgq, "cgk": cgk, "sgk": sgk,
        })
    return in_maps


_CACHE = {}


def run_full(inputs, trace=False, **kw):
    if "nc" not in _CACHE:
        _CACHE["nc"] = build_program(S)
    nc = _CACHE["nc"]
    in_maps = prep_inputs(inputs, S)
    res = bass_utils.run_bass_kernel_spmd(
        nc, in_maps, core_ids=list(range(NCORES)), trace=trace, **kw)
    total = res.results[0]["out"].astype(np.float64)
    for c in range(1, NCORES):
        total += res.results[c]["out"]
    total += np.asarray(inputs["bo"], np.float64)[None, :]
    out = total.astype(np.float32).reshape(1, S, D)
    return out, res


def kernel(**inputs):
    out, _ = run_full(inputs, trace=False)
    return out
